# revision 4
# baseline (speedup 1.0000x reference)
"""Trainium2 Bass kernel for MHA with ALiBi + causal mask.

Problem: B=2, S=2048, D_MODEL=2048, H=16, HEAD_DIM=128, fp32 I/O.
Sharding: tensor-parallel over heads — core c owns heads [2c, 2c+2) for both
batches. x is shipped sharded (1/8 per core) and AllGathered on device; each
core computes its heads' Q/K/V projections, attention, and a rank-256 partial
of the output projection; a ReduceScatter sums the partials so each core
returns a disjoint 512-row slice of y in fp16.

Wire-format choices (the axon tunnel is the bottleneck, ~50MB/s):
  x, W: fp16 (matmul precision; fp8 would blow the 2e-2 error gate since
        dot-product relative error does not average down over random signs)
  alibi: int8 with a fixed dequant scale — only the causally-needed lower
        triangle is shipped, packed at [128k x 512q] tile granularity with
        ragged diagonal tiles (53% of the full tensor). The intra-tile causal
        mask is applied on device via gpsimd.affine_select, so masked regions
        never cross the wire and may hold garbage.
  y: fp16 out (plus the donated zero buffers shipped in).

Device pipeline per core:
  AllGather xT (fp8-free, fp16) -> DRAM reorder to [p, ec, s]
  phase 1: Q^T,K^T (weights stationary) and V natural (x stationary), fp16
  phase 2: scores^T = K @ Q^T per 128x512 block; int8 alibi dequant fused
           into the PSUM bias add (scalar_tensor_tensor); causal fill via
           affine_select on diagonal tiles; exp on ScalarE; denominators via
           ones-vector matmul; PV accumulation (out^T layout); normalize via
           reciprocal broadcast matmul
  phase 3: partial output projection -> fp16 DRAM -> ReduceScatter(add)
"""

import numpy as np

D_MODEL = 2048
N_HEADS = 16
HEAD_DIM = 128
BATCH = 2
SEQ = 2048
N_CORES = 8
H_LOC = 2          # heads per core
EC = 16            # 128-row chunks of the d_model contraction dim
SC = 512           # s-chunk (matmul free dim)
BS = BATCH * SEQ   # 4096
NEG = -240.0       # causal fill after dequant, exp -> 0
S_ALIBI = 0.6 / 127.0   # fixed int8 dequant scale for the alibi bias

# packed-alibi column offsets: per q-block qj, 4*qj full [128,512] tiles then
# 4 ragged diagonal tiles of widths 512,384,256,128
DIAG_OFF = [0, 512, 896, 1152]
AL_QOFF = [0, 1280, 4608, 9984]
AL_COLS = 17408

_cache = {}


def _build():
    import concourse.mybir as mybir
    from concourse import bacc
    import concourse.tile as tile

    FP16 = mybir.dt.float16
    F32 = mybir.dt.float32
    I8 = mybir.dt.int8
    P = 128

    nc = bacc.Bacc(None, target_bir_lowering=False)

    xs_d = nc.dram_tensor("xs", [H_LOC, P, BS], FP16, kind="ExternalInput")
    wq_d = nc.dram_tensor("wqT", [P, EC, H_LOC * HEAD_DIM], FP16, kind="ExternalInput")
    wk_d = nc.dram_tensor("wkT", [P, EC, H_LOC * HEAD_DIM], FP16, kind="ExternalInput")
    wv_d = nc.dram_tensor("wvT", [P, EC, H_LOC * HEAD_DIM], FP16, kind="ExternalInput")
    wo_d = nc.dram_tensor("woT", [P, H_LOC, D_MODEL], FP16, kind="ExternalInput")
    al_d = nc.dram_tensor("alibi8", [H_LOC, P, AL_COLS], I8, kind="ExternalInput")
    y_d = nc.dram_tensor("y", [BS // P // N_CORES, P, D_MODEL], FP16,
                         kind="ExternalOutput")

    mult = mybir.AluOpType.mult
    add = mybir.AluOpType.add
    Exp = mybir.ActivationFunctionType.Exp
    GROUP = [list(range(N_CORES))]

    with tile.TileContext(nc) as tc:
        with tc.tile_pool(name="dram", bufs=1, space="DRAM") as dram, \
             tc.tile_pool(name="const", bufs=1) as constp, \
             tc.tile_pool(name="wpool", bufs=1) as wpool, \
             tc.tile_pool(name="qkv", bufs=1) as qkvp, \
             tc.tile_pool(name="xp", bufs=2) as xp, \
             tc.tile_pool(name="attn", bufs=4) as apool, \
             tc.tile_pool(name="ali", bufs=2) as bpool, \
             tc.tile_pool(name="rcp", bufs=4) as rcpool, \
             tc.tile_pool(name="rbp", bufs=2) as rbpool, \
             tc.tile_pool(name="yp", bufs=4) as ypool:

            # ---- AllGather x across cores, then reorder to [p, ec, s] ----
            xin = dram.tile([H_LOC, P, BS], FP16)
            xg = dram.tile([EC, P, BS], FP16)
            xg2 = dram.tile([P, EC, BS], FP16)
            nc.gpsimd.dma_start(xin[:], xs_d[:])
            nc.gpsimd.collective_compute(
                "AllGather", mybir.AluOpType.bypass,
                replica_groups=GROUP, ins=[xin.opt()], outs=[xg.opt()])
            for e in range(EC):
                nc.gpsimd.dma_start(xg2[:, e, :], xg[e, :, :])

            yp_dram = dram.tile([BS // P, P, D_MODEL], FP16)
            yb = dram.tile([BS // P // N_CORES, P, D_MODEL], FP16)

            ones = constp.tile([P, 1], FP16, tag="ones", name="ones")
            nc.vector.memset(ones, 1.0)
            ones1 = constp.tile([1, P], F32, tag="ones1", name="ones1")
            nc.vector.memset(ones1, 1.0)

            wq = wpool.tile([P, EC, 256], FP16, tag="wq", name="wq")
            wk = wpool.tile([P, EC, 256], FP16, tag="wk", name="wk")
            wv = wpool.tile([P, EC, 256], FP16, tag="wv", name="wv")
            wo = wpool.tile([P, H_LOC, D_MODEL], FP16, tag="wo", name="wo")
            nc.sync.dma_start(out=wq, in_=wq_d[:, :, :])
            nc.sync.dma_start(out=wk, in_=wk_d[:, :, :])
            nc.sync.dma_start(out=wv, in_=wv_d[:, :, :])
            nc.sync.dma_start(out=wo, in_=wo_d[:, :, :])

            # persistent per-(batch, head) activations, fp16
            QT = [[qkvp.tile([P, SEQ], FP16, tag=f"q{b}{h}", name=f"q{b}{h}") for h in range(2)]
                  for b in range(2)]
            KT = [[qkvp.tile([P, SEQ], FP16, tag=f"k{b}{h}", name=f"k{b}{h}") for h in range(2)]
                  for b in range(2)]
            V = [qkvp.tile([P, EC, 256], FP16, tag=f"v{b}", name=f"v{b}") for b in range(2)]
            OT = [[qkvp.tile([P, SEQ], FP16, tag=f"o{b}{h}", name=f"o{b}{h}") for h in range(2)]
                  for b in range(2)]

            # ---- phase 1: projections ----
            with tc.tile_pool(name="ps1", bufs=4, space="PSUM") as ps_qk, \
                 tc.tile_pool(name="ps1v", bufs=3, space="PSUM") as ps_v:
                for c8 in range(BS // SC):          # 8 chunks of 512 rows of x
                    b, scn = c8 // 4, c8 % 4
                    xt = xp.tile([P, EC, SC], FP16, tag="xt", name="xt")
                    nc.sync.dma_start(
                        out=xt, in_=xg2[:, :, c8 * SC:(c8 + 1) * SC])
                    for W_sb, dest in ((wq, QT), (wk, KT)):
                        for h in range(2):
                            ps = ps_qk.tile([P, SC], F32, tag="qk", name="qk")
                            for e in range(EC):
                                nc.tensor.matmul(
                                    ps,
                                    lhsT=W_sb[:, e, h * P:(h + 1) * P],
                                    rhs=xt[:, e, :],
                                    start=(e == 0), stop=(e == EC - 1))
                            nc.scalar.copy(
                                out=dest[b][h][:, scn * SC:(scn + 1) * SC], in_=ps)
                    for st in range(SC // P):       # V natural, 4 tiles of 128
                        psv = ps_v.tile([P, 256], F32, tag="v")
                        for e in range(EC):
                            nc.tensor.matmul(
                                psv,
                                lhsT=xt[:, e, st * P:(st + 1) * P],
                                rhs=wv[:, e, :],
                                start=(e == 0), stop=(e == EC - 1))
                        tv = scn * 4 + st
                        nc.scalar.copy(out=V[b][:, tv, :], in_=psv)

            # ---- phase 2: attention ----
            with tc.tile_pool(name="ps2s", bufs=3, space="PSUM") as ps_sc, \
                 tc.tile_pool(name="ps2o", bufs=2, space="PSUM") as ps_out, \
                 tc.tile_pool(name="ps2m", bufs=2, space="PSUM") as ps_sum, \
                 tc.tile_pool(name="ps2b", bufs=1, space="PSUM") as ps_bc:
                for h in range(2):
                    for qj in range(SEQ // SC):     # 4 query chunks of 512
                        nkt = 4 * qj + 4            # causal: k tiles 0..4qj+3
                        qoff = AL_QOFF[qj]
                        if qj:
                            slab = bpool.tile([P, 6144], I8, tag="alf",
                                              name="alf")
                            nc.sync.dma_start(
                                out=slab[:, :4 * qj * SC],
                                in_=al_d[h, :, qoff:qoff + 4 * qj * SC])
                        adiag = bpool.tile([P, 4, SC], I8, tag="ald",
                                           name="ald")
                        for t in range(4):
                            w = SC - t * P
                            doff = qoff + 4 * qj * SC + DIAG_OFF[t]
                            nc.sync.dma_start(
                                out=adiag[:, t, t * P:],
                                in_=al_d[h, :, doff:doff + w])
                        out_ps = [ps_out.tile([P, SC], F32, tag="out", name="out")
                                  for _ in range(2)]
                        sum_ps = [ps_sum.tile([1, SC], F32, tag="sum", name="sum")
                                  for _ in range(2)]
                        for ki in range(nkt):
                            t = ki - 4 * qj
                            if t < 0:
                                a_sl = slab[:, ki * SC:(ki + 1) * SC]
                            else:
                                a_sl = adiag[:, t, :]
                            for b in range(2):
                                sc_ps = ps_sc.tile([P, SC], F32, tag="sc", name="sc")
                                nc.tensor.matmul(
                                    sc_ps,
                                    lhsT=KT[b][h][:, ki * P:(ki + 1) * P],
                                    rhs=QT[b][h][:, qj * SC:(qj + 1) * SC],
                                    start=True, stop=True)
                                at32 = apool.tile([P, SC], F32, tag="at32",
                                                  name="at32")
                                nc.vector.scalar_tensor_tensor(
                                    out=at32, in0=a_sl, scalar=S_ALIBI,
                                    in1=sc_ps, op0=mult, op1=add)
                                if t >= 0:
                                    # causal: keep where q >= k, i.e. c >= p + t*128
                                    nc.gpsimd.affine_select(
                                        out=at32, in_=at32,
                                        compare_op=mybir.AluOpType.is_ge,
                                        fill=NEG, base=-(t * P),
                                        pattern=[[1, SC]],
                                        channel_multiplier=-1)
                                at = apool.tile([P, SC], FP16, tag="at", name="at")
                                nc.scalar.activation(at, at32, Exp)
                                nc.tensor.matmul(sum_ps[b], lhsT=ones, rhs=at,
                                                 start=(ki == 0),
                                                 stop=(ki == nkt - 1))
                                nc.tensor.matmul(
                                    out_ps[b],
                                    lhsT=V[b][:, ki, h * P:(h + 1) * P],
                                    rhs=at,
                                    start=(ki == 0), stop=(ki == nkt - 1))
                        for b in range(2):
                            rc = rcpool.tile([1, SC], F32, tag="rc", name="rc")
                            nc.vector.reciprocal(out=rc, in_=sum_ps[b])
                            bc = ps_bc.tile([P, SC], F32, tag="bc", name="bc")
                            nc.tensor.matmul(bc, lhsT=ones1, rhs=rc,
                                             start=True, stop=True)
                            rb = rbpool.tile([P, SC], F32, tag="rb", name="rb")
                            nc.scalar.copy(out=rb, in_=bc)
                            nc.vector.scalar_tensor_tensor(
                                out=OT[b][h][:, qj * SC:(qj + 1) * SC],
                                in0=out_ps[b], scalar=1.0, in1=rb,
                                op0=mult, op1=mult)

            # ---- phase 3: output projection partial -> DRAM fp16 ----
            with tc.tile_pool(name="ps3", bufs=4, space="PSUM") as ps_y:
                for b in range(2):
                    for st in range(SEQ // P):      # 16 row tiles per batch
                        ysb = ypool.tile([P, D_MODEL], FP16, tag="ysb",
                                         name="ysb")
                        for mj in range(D_MODEL // SC):
                            yps = ps_y.tile([P, SC], F32, tag="y", name="y")
                            for h in range(2):
                                nc.tensor.matmul(
                                    yps,
                                    lhsT=OT[b][h][:, st * P:(st + 1) * P],
                                    rhs=wo[:, h, mj * SC:(mj + 1) * SC],
                                    start=(h == 0), stop=(h == 1))
                            if mj % 2 == 0:
                                nc.scalar.copy(
                                    out=ysb[:, mj * SC:(mj + 1) * SC], in_=yps)
                            else:
                                nc.vector.tensor_copy(
                                    out=ysb[:, mj * SC:(mj + 1) * SC], in_=yps)
                        nc.sync.dma_start(out=yp_dram[b * 16 + st, :, :],
                                          in_=ysb)

            # ---- ReduceScatter the rank-256 partials; core c gets rows
            # [c*512, (c+1)*512) of y fully summed ----
            nc.gpsimd.collective_compute(
                "ReduceScatter", add,
                replica_groups=GROUP, ins=[yp_dram.opt()], outs=[yb.opt()])
            nc.gpsimd.dma_start(y_d[:], yb[:])
    nc.compile()
    return nc


def _install_compile_cache(nc):
    """Memoize the walrus NEFF build (a pure function of the BIR bytes).

    The bass_exec path bypasses the platform's neuron compile cache, so
    every run_bass_kernel_spmd call re-runs walrus (~0.25s) on an identical
    BIR. Cache it keyed on the BIR hash and pre-populate for the main
    kernel so the first timed run skips it too.
    """
    import hashlib, tempfile
    import concourse.bass2jax as b2j
    from concourse.bass_utils import compile_bir_kernel as _orig

    cache = _cache.setdefault("neff_cache", {})

    def _cached(bir_json, tmpdir, neff_name="file.neff"):
        bb = bir_json if isinstance(bir_json, bytes) else bir_json.encode()
        key = hashlib.sha256(bb).hexdigest()
        hit = cache.get(key)
        if hit is None:
            # persistent dir: the neff file is re-read on later cache hits
            hit = _orig(bir_json, tempfile.mkdtemp(), neff_name=neff_name)
            cache[key] = hit
        return hit

    b2j.compile_bir_kernel = _cached
    _cached(nc.to_json_bytes(), None)


def _build_warmup():
    """Tiny kernel exercising the collective path: absorbs one-time axon
    terminal init (device bring-up, global comm build) into untimed prep."""
    import concourse.mybir as mybir
    from concourse import bacc
    import concourse.tile as tile

    F32 = mybir.dt.float32
    nc = bacc.Bacc(None, target_bir_lowering=False)
    in_d = nc.dram_tensor("win", [128, 8], F32, kind="ExternalInput")
    out_d = nc.dram_tensor("wout", [128, 8], F32, kind="ExternalOutput")
    with tile.TileContext(nc) as tc:
        with tc.tile_pool(name="dram", bufs=1, space="DRAM") as dram:
            bin_ = dram.tile([128, 8], F32)
            agg = dram.tile([N_CORES, 128, 8], F32)
            rs = dram.tile([128, 8], F32)
            nc.gpsimd.dma_start(bin_[:], in_d[:])
            nc.gpsimd.collective_compute(
                "AllGather", mybir.AluOpType.bypass,
                replica_groups=[list(range(N_CORES))],
                ins=[bin_.opt()], outs=[agg.opt()])
            nc.gpsimd.collective_compute(
                "ReduceScatter", mybir.AluOpType.add,
                replica_groups=[list(range(N_CORES))],
                ins=[agg.opt()], outs=[rs.opt()])
            nc.gpsimd.dma_start(out_d[:], rs[:])
    nc.compile()
    return nc


def _pack_alibi(A_h):
    """[q, k] f32 head slice -> [128, AL_COLS] int8 causal-packed."""
    q8 = np.clip(np.rint(A_h.T * (1.0 / S_ALIBI)), -127, 127).astype(np.int8)
    T3 = np.ascontiguousarray(q8).reshape(EC, 128, SEQ)   # [ki, p, q]
    segs = []
    for qj in range(4):
        qs = slice(qj * SC, (qj + 1) * SC)
        if qj:
            segs.append(T3[:4 * qj, :, qs].transpose(1, 0, 2).reshape(128, -1))
        for t in range(4):
            segs.append(T3[4 * qj + t, :, qj * SC + t * 128:(qj + 1) * SC])
    return np.concatenate(segs, axis=1)


def _prep_inputs(x, alibi_bias, W_q, W_k, W_v, W_o):
    f16 = np.float16
    x = np.asarray(x, np.float32).reshape(BS, D_MODEL)
    # xT[e, s] -> [ec, p, s] fp16; core c ships ec chunks [2c, 2c+2)
    xT = x.T.astype(f16).reshape(EC, 128, BS)

    scale = 1.0 / np.sqrt(np.float32(HEAD_DIM))

    in_maps = []
    for c in range(N_CORES):
        rows = slice(c * 256, (c + 1) * 256)

        def wt(W, s=1.0):
            # [e=2048, d_loc=256] -> [p, e_chunk, d]
            wT = (np.asarray(W, np.float32)[rows] * s).T
            return np.ascontiguousarray(
                wT.reshape(EC, 128, 256).transpose(1, 0, 2).astype(f16))

        woT = np.asarray(W_o, np.float32)[:, rows].T      # [256, 2048]
        woT = np.ascontiguousarray(
            woT.reshape(H_LOC, 128, D_MODEL).transpose(1, 0, 2).astype(f16))

        alibi8 = np.stack([
            _pack_alibi(np.asarray(alibi_bias[2 * c + hl], np.float32))
            for hl in range(H_LOC)])

        in_maps.append({
            "xs": np.ascontiguousarray(xT[2 * c:2 * c + 2]),
            "wqT": wt(W_q, scale),
            "wkT": wt(W_k),
            "wvT": wt(W_v),
            "woT": woT,
            "alibi8": alibi8,
        })
    return in_maps


def kernel(x, alibi_bias, W_q, W_k, W_v, W_o, _trace=False):
    import time as _time
    from concourse.bass_utils import run_bass_kernel_spmd

    if "nc" not in _cache:
        _cache["nc"] = _build()
        _install_compile_cache(_cache["nc"])
    nc = _cache["nc"]

    t0 = _time.time()
    if not _cache.get("warmed"):
        wnc = _build_warmup()
        wmaps = [{"win": np.zeros((128, 8), np.float32)} for _ in range(N_CORES)]
        run_bass_kernel_spmd(wnc, wmaps, core_ids=list(range(N_CORES)))
        _cache["warmed"] = True
    in_maps = _prep_inputs(x, alibi_bias, W_q, W_k, W_v, W_o)
    _cache["prep_s"] = _time.time() - t0
    t0 = _time.time()
    res = run_bass_kernel_spmd(nc, in_maps, core_ids=list(range(N_CORES)),
                               trace=_trace)
    _cache["run_s"] = _time.time() - t0
    _cache["last_result"] = res
    y16 = np.concatenate(
        [np.asarray(om["y"], np.float16).reshape(SEQ // 4, D_MODEL)
         for om in res.results], axis=0)
    return y16.astype(np.float32).reshape(BATCH, SEQ, D_MODEL)


# revision 16
# speedup vs baseline: 1.2336x; 1.2336x over previous
"""Trainium2 Bass kernel for MHA with ALiBi + causal mask.

Problem: B=2, S=2048, D_MODEL=2048, H=16, HEAD_DIM=128, fp32 I/O.
Sharding: tensor-parallel over heads — core c owns heads [2c, 2c+2) for both
batches. x is shipped sharded (1/8 per core) and AllGathered on device; each
core computes its heads' Q/K/V projections, attention, and a rank-256 partial
of the output projection; a ReduceScatter sums the partials so each core
returns a disjoint 512-row slice of y in fp16.

Wire-format choices (the axon tunnel is the bottleneck, ~50MB/s):
  x, W: fp16 (matmul precision; fp8 would blow the 2e-2 error gate since
        dot-product relative error does not average down over random signs)
  alibi: int8 with a fixed dequant scale — only the causally-needed lower
        triangle is shipped, packed at [128k x 512q] tile granularity with
        ragged diagonal tiles (53% of the full tensor). The intra-tile causal
        mask is applied on device via gpsimd.affine_select, so masked regions
        never cross the wire and may hold garbage.
  y: fp16 out (plus the donated zero buffers shipped in).

Device pipeline per core:
  AllGather xT (fp8-free, fp16) -> DRAM reorder to [p, ec, s]
  phase 1: Q^T,K^T (weights stationary) and V natural (x stationary), fp16
  phase 2: scores^T = K @ Q^T per 128x512 block; int8 alibi dequant fused
           into the PSUM bias add (scalar_tensor_tensor); causal fill via
           affine_select on diagonal tiles; exp on ScalarE; denominators via
           ones-vector matmul; PV accumulation (out^T layout); normalize via
           reciprocal broadcast matmul
  phase 3: partial output projection -> fp16 DRAM -> ReduceScatter(add)
"""

import numpy as np

D_MODEL = 2048
N_HEADS = 16
HEAD_DIM = 128
BATCH = 2
SEQ = 2048
N_CORES = 8
H_LOC = 2          # heads per core
EC = 16            # 128-row chunks of the d_model contraction dim
SC = 512           # s-chunk (matmul free dim)
BS = BATCH * SEQ   # 4096
NEG = -240.0       # causal fill after dequant, exp -> 0
S_ALIBI = 0.6 / 127.0   # fixed int8 dequant scale for the alibi bias

# packed-alibi column offsets: per q-block qj, 4*qj full [128,512] tiles then
# 4 ragged diagonal tiles of widths 512,384,256,128
DIAG_OFF = [0, 512, 896, 1152]
AL_QOFF = [0, 1280, 4608, 9984]
AL_COLS = 17408

_cache = {}


def _build():
    import concourse.mybir as mybir
    from concourse import bacc
    import concourse.tile as tile

    FP16 = mybir.dt.float16
    F32 = mybir.dt.float32
    I8 = mybir.dt.int8
    U8 = mybir.dt.uint8
    U16 = mybir.dt.uint16
    P = 128
    shl = mybir.AluOpType.logical_shift_left
    band = mybir.AluOpType.bitwise_and
    bor = mybir.AluOpType.bitwise_or

    nc = bacc.Bacc(None, target_bir_lowering=False)

    # x and W ship as 12-bit floats: a hi-byte plane plus a plane of packed
    # mantissa nibbles (pair j with j+H along the last dim)
    xs_d = nc.dram_tensor("xs", [H_LOC, P, BS + BS // 2], U8, kind="ExternalInput")
    wq_d = nc.dram_tensor("wqT", [P, EC, 384], U8, kind="ExternalInput")
    wk_d = nc.dram_tensor("wkT", [P, EC, 384], U8, kind="ExternalInput")
    wv_d = nc.dram_tensor("wvT", [P, EC, 384], U8, kind="ExternalInput")
    wo_d = nc.dram_tensor("woT", [P, 8, 768], U8, kind="ExternalInput")
    al_d = nc.dram_tensor("alibi8", [H_LOC, P, AL_COLS], I8, kind="ExternalInput")
    y_d = nc.dram_tensor("y", [BS // P // N_CORES, P, D_MODEL], FP16,
                         kind="ExternalOutput")

    def widen12(hi, lo, hi16, lo16a, lo16b):
        """hi16 <- hi<<8; lo16a <- lo&0xF0; lo16b <- (lo&0x0F)<<4 (all u16).
        Caller ORs hi16 halves with lo16a/lo16b into the fp16 target."""
        nc.scalar.copy(out=hi16, in_=hi)
        nc.vector.tensor_scalar(out=hi16, in0=hi16, scalar1=8, scalar2=None,
                                op0=shl)
        nc.scalar.copy(out=lo16a, in_=lo)
        nc.scalar.copy(out=lo16b, in_=lo)
        nc.vector.tensor_scalar(out=lo16a, in0=lo16a, scalar1=0xF0,
                                scalar2=None, op0=band)
        nc.vector.tensor_scalar(out=lo16b, in0=lo16b, scalar1=0x0F, scalar2=4,
                                op0=band, op1=shl)

    mult = mybir.AluOpType.mult
    add = mybir.AluOpType.add
    Exp = mybir.ActivationFunctionType.Exp
    GROUP = [list(range(N_CORES))]

    with tile.TileContext(nc) as tc:
        with tc.tile_pool(name="dram", bufs=1, space="DRAM") as dram, \
             tc.tile_pool(name="const", bufs=1) as constp, \
             tc.tile_pool(name="wpool", bufs=1) as wpool, \
             tc.tile_pool(name="qkv", bufs=1) as qkvp, \
             tc.tile_pool(name="xp", bufs=2) as xp, \
             tc.tile_pool(name="attn", bufs=4) as apool, \
             tc.tile_pool(name="ali", bufs=2) as bpool, \
             tc.tile_pool(name="rcp", bufs=4) as rcpool, \
             tc.tile_pool(name="rbp", bufs=2) as rbpool, \
             tc.tile_pool(name="yp", bufs=4) as ypool:

            # ---- AllGather packed x across cores, split/reorder to [p, ec, s] ----
            xin = dram.tile([H_LOC, P, BS + BS // 2], U8)
            xg = dram.tile([EC, P, BS + BS // 2], U8)
            xhi2 = dram.tile([P, EC, BS], U8)
            xlo2 = dram.tile([P, EC, BS // 2], U8)
            nc.gpsimd.dma_start(xin[:], xs_d[:])
            nc.gpsimd.collective_compute(
                "AllGather", mybir.AluOpType.bypass,
                replica_groups=GROUP, ins=[xin.opt()], outs=[xg.opt()])
            for e in range(EC):
                nc.gpsimd.dma_start(xhi2[:, e, :], xg[e, :, :BS])
                nc.gpsimd.dma_start(xlo2[:, e, :], xg[e, :, BS:])

            yp_dram = dram.tile([BS // P, P, D_MODEL], FP16)
            yb = dram.tile([BS // P // N_CORES, P, D_MODEL], FP16)

            ones = constp.tile([P, 1], FP16, tag="ones", name="ones")
            nc.vector.memset(ones, 1.0)
            ones1 = constp.tile([1, P], F32, tag="ones1", name="ones1")
            nc.vector.memset(ones1, 1.0)

            wq = wpool.tile([P, EC, 256], FP16, tag="wq", name="wq")
            wk = wpool.tile([P, EC, 256], FP16, tag="wk", name="wk")
            wv = wpool.tile([P, EC, 256], FP16, tag="wv", name="wv")
            wo = wpool.tile([P, H_LOC, D_MODEL], FP16, tag="wo", name="wo")
            def wo_seg(s):
                return wo[:, s // 4, (s % 4) * SC:(s % 4 + 1) * SC]

            with tc.tile_pool(name="wup", bufs=2) as wup:
                for W_d, seg_of, nseg, L in (
                        (wq_d, lambda s: wq[:, s, :], EC, 256),
                        (wk_d, lambda s: wk[:, s, :], EC, 256),
                        (wv_d, lambda s: wv[:, s, :], EC, 256),
                        (wo_d, wo_seg, 8, SC)):
                    H = L // 2
                    for sg in range(nseg):
                        whi = wup.tile([P, SC], U8, tag="whi", name="whi")
                        wlo = wup.tile([P, SC // 2], U8, tag="wlo", name="wlo")
                        nc.sync.dma_start(out=whi[:, :L], in_=W_d[:, sg, :L])
                        nc.sync.dma_start(out=wlo[:, :H], in_=W_d[:, sg, L:])
                        hi16 = wup.tile([P, SC], U16, tag="hi16", name="hi16")
                        l16a = wup.tile([P, SC // 2], U16, tag="l16a", name="l16a")
                        l16b = wup.tile([P, SC // 2], U16, tag="l16b", name="l16b")
                        widen12(whi[:, :L], wlo[:, :H], hi16[:, :L],
                                l16a[:, :H], l16b[:, :H])
                        tgt = seg_of(sg)
                        nc.vector.tensor_tensor(
                            out=tgt[:, :H].bitcast(U16),
                            in0=hi16[:, :H], in1=l16a[:, :H], op=bor)
                        nc.vector.tensor_tensor(
                            out=tgt[:, H:].bitcast(U16),
                            in0=hi16[:, H:L], in1=l16b[:, :H], op=bor)

            # persistent per-(batch, head) activations, fp16
            QT = [[qkvp.tile([P, SEQ], FP16, tag=f"q{b}{h}", name=f"q{b}{h}") for h in range(2)]
                  for b in range(2)]
            KT = [[qkvp.tile([P, SEQ], FP16, tag=f"k{b}{h}", name=f"k{b}{h}") for h in range(2)]
                  for b in range(2)]
            V = [qkvp.tile([P, EC, 256], FP16, tag=f"v{b}", name=f"v{b}") for b in range(2)]
            OT = [[qkvp.tile([P, SEQ], FP16, tag=f"o{b}{h}", name=f"o{b}{h}") for h in range(2)]
                  for b in range(2)]

            # ---- phase 1: projections ----
            with tc.tile_pool(name="ps1", bufs=4, space="PSUM") as ps_qk, \
                 tc.tile_pool(name="ps1v", bufs=3, space="PSUM") as ps_v, \
                 tc.tile_pool(name="xup", bufs=1) as xup:
                for c8 in range(BS // SC):          # 8 chunks of 512 rows of x
                    b, scn = c8 // 4, c8 % 4
                    HX = SC // 2
                    xt = xp.tile([P, EC, SC], FP16, tag="xt", name="xt")
                    for eg in range(0, EC, 4):      # unpack 4 e-chunks at a time
                        xth = xup.tile([P, 4, SC], U8, tag="xth", name="xth")
                        xtl = xup.tile([P, 4, HX], U8, tag="xtl", name="xtl")
                        nc.sync.dma_start(
                            out=xth,
                            in_=xhi2[:, eg:eg + 4, c8 * SC:(c8 + 1) * SC])
                        nc.sync.dma_start(
                            out=xtl,
                            in_=xlo2[:, eg:eg + 4, c8 * HX:(c8 + 1) * HX])
                        hi16 = xup.tile([P, 4, SC], U16, tag="xhi16", name="xhi16")
                        l16a = xup.tile([P, 4, HX], U16, tag="xl16a", name="xl16a")
                        l16b = xup.tile([P, 4, HX], U16, tag="xl16b", name="xl16b")
                        widen12(xth, xtl, hi16, l16a, l16b)
                        nc.vector.tensor_tensor(
                            out=xt[:, eg:eg + 4, :HX].bitcast(U16),
                            in0=hi16[:, :, :HX], in1=l16a, op=bor)
                        nc.vector.tensor_tensor(
                            out=xt[:, eg:eg + 4, HX:].bitcast(U16),
                            in0=hi16[:, :, HX:], in1=l16b, op=bor)
                    for W_sb, dest in ((wq, QT), (wk, KT)):
                        for h in range(2):
                            ps = ps_qk.tile([P, SC], F32, tag="qk", name="qk")
                            for e in range(EC):
                                nc.tensor.matmul(
                                    ps,
                                    lhsT=W_sb[:, e, h * P:(h + 1) * P],
                                    rhs=xt[:, e, :],
                                    start=(e == 0), stop=(e == EC - 1))
                            nc.scalar.copy(
                                out=dest[b][h][:, scn * SC:(scn + 1) * SC], in_=ps)
                    for st in range(SC // P):       # V natural, 4 tiles of 128
                        psv = ps_v.tile([P, 256], F32, tag="v")
                        for e in range(EC):
                            nc.tensor.matmul(
                                psv,
                                lhsT=xt[:, e, st * P:(st + 1) * P],
                                rhs=wv[:, e, :],
                                start=(e == 0), stop=(e == EC - 1))
                        tv = scn * 4 + st
                        nc.scalar.copy(out=V[b][:, tv, :], in_=psv)

            # ---- phase 2: attention ----
            with tc.tile_pool(name="ps2s", bufs=3, space="PSUM") as ps_sc, \
                 tc.tile_pool(name="ps2o", bufs=2, space="PSUM") as ps_out, \
                 tc.tile_pool(name="ps2m", bufs=2, space="PSUM") as ps_sum, \
                 tc.tile_pool(name="ps2b", bufs=1, space="PSUM") as ps_bc:
                for h in range(2):
                    for qj in range(SEQ // SC):     # 4 query chunks of 512
                        nkt = 4 * qj + 4            # causal: k tiles 0..4qj+3
                        qoff = AL_QOFF[qj]
                        if qj:
                            slab = bpool.tile([P, 6144], I8, tag="alf",
                                              name="alf")
                            nc.sync.dma_start(
                                out=slab[:, :4 * qj * SC],
                                in_=al_d[h, :, qoff:qoff + 4 * qj * SC])
                        adiag = bpool.tile([P, 4, SC], I8, tag="ald",
                                           name="ald")
                        for t in range(4):
                            w = SC - t * P
                            doff = qoff + 4 * qj * SC + DIAG_OFF[t]
                            nc.sync.dma_start(
                                out=adiag[:, t, t * P:],
                                in_=al_d[h, :, doff:doff + w])
                        out_ps = [ps_out.tile([P, SC], F32, tag="out", name="out")
                                  for _ in range(2)]
                        sum_ps = [ps_sum.tile([1, SC], F32, tag="sum", name="sum")
                                  for _ in range(2)]
                        for ki in range(nkt):
                            t = ki - 4 * qj
                            if t < 0:
                                a_sl = slab[:, ki * SC:(ki + 1) * SC]
                            else:
                                a_sl = adiag[:, t, :]
                            for b in range(2):
                                sc_ps = ps_sc.tile([P, SC], F32, tag="sc", name="sc")
                                nc.tensor.matmul(
                                    sc_ps,
                                    lhsT=KT[b][h][:, ki * P:(ki + 1) * P],
                                    rhs=QT[b][h][:, qj * SC:(qj + 1) * SC],
                                    start=True, stop=True)
                                at32 = apool.tile([P, SC], F32, tag="at32",
                                                  name="at32")
                                nc.vector.scalar_tensor_tensor(
                                    out=at32, in0=a_sl, scalar=S_ALIBI,
                                    in1=sc_ps, op0=mult, op1=add)
                                if t >= 0:
                                    # causal: keep where q >= k, i.e. c >= p + t*128
                                    nc.gpsimd.affine_select(
                                        out=at32, in_=at32,
                                        compare_op=mybir.AluOpType.is_ge,
                                        fill=NEG, base=-(t * P),
                                        pattern=[[1, SC]],
                                        channel_multiplier=-1)
                                at = apool.tile([P, SC], FP16, tag="at", name="at")
                                nc.scalar.activation(at, at32, Exp)
                                nc.tensor.matmul(sum_ps[b], lhsT=ones, rhs=at,
                                                 start=(ki == 0),
                                                 stop=(ki == nkt - 1))
                                nc.tensor.matmul(
                                    out_ps[b],
                                    lhsT=V[b][:, ki, h * P:(h + 1) * P],
                                    rhs=at,
                                    start=(ki == 0), stop=(ki == nkt - 1))
                        for b in range(2):
                            rc = rcpool.tile([1, SC], F32, tag="rc", name="rc")
                            nc.vector.reciprocal(out=rc, in_=sum_ps[b])
                            bc = ps_bc.tile([P, SC], F32, tag="bc", name="bc")
                            nc.tensor.matmul(bc, lhsT=ones1, rhs=rc,
                                             start=True, stop=True)
                            rb = rbpool.tile([P, SC], F32, tag="rb", name="rb")
                            nc.scalar.copy(out=rb, in_=bc)
                            nc.vector.scalar_tensor_tensor(
                                out=OT[b][h][:, qj * SC:(qj + 1) * SC],
                                in0=out_ps[b], scalar=1.0, in1=rb,
                                op0=mult, op1=mult)

            # ---- phase 3: output projection partial -> DRAM fp16 ----
            with tc.tile_pool(name="ps3", bufs=4, space="PSUM") as ps_y:
                for b in range(2):
                    for st in range(SEQ // P):      # 16 row tiles per batch
                        ysb = ypool.tile([P, D_MODEL], FP16, tag="ysb",
                                         name="ysb")
                        for mj in range(D_MODEL // SC):
                            yps = ps_y.tile([P, SC], F32, tag="y", name="y")
                            for h in range(2):
                                nc.tensor.matmul(
                                    yps,
                                    lhsT=OT[b][h][:, st * P:(st + 1) * P],
                                    rhs=wo[:, h, mj * SC:(mj + 1) * SC],
                                    start=(h == 0), stop=(h == 1))
                            if mj % 2 == 0:
                                nc.scalar.copy(
                                    out=ysb[:, mj * SC:(mj + 1) * SC], in_=yps)
                            else:
                                nc.vector.tensor_copy(
                                    out=ysb[:, mj * SC:(mj + 1) * SC], in_=yps)
                        nc.sync.dma_start(out=yp_dram[b * 16 + st, :, :],
                                          in_=ysb)

            # ---- ReduceScatter the rank-256 partials; core c gets rows
            # [c*512, (c+1)*512) of y fully summed ----
            nc.gpsimd.collective_compute(
                "ReduceScatter", add,
                replica_groups=GROUP, ins=[yp_dram.opt()], outs=[yb.opt()])
            nc.gpsimd.dma_start(y_d[:], yb[:])
    nc.compile()
    return nc


def _install_compile_cache(nc):
    """Memoize the walrus NEFF build (a pure function of the BIR bytes).

    The bass_exec path bypasses the platform's neuron compile cache, so
    every run_bass_kernel_spmd call re-runs walrus (~0.25s) on an identical
    BIR. Cache it keyed on the BIR hash and pre-populate for the main
    kernel so the first timed run skips it too.
    """
    import hashlib, tempfile
    import concourse.bass2jax as b2j
    from concourse.bass_utils import compile_bir_kernel as _orig

    cache = _cache.setdefault("neff_cache", {})

    def _cached(bir_json, tmpdir, neff_name="file.neff"):
        bb = bir_json if isinstance(bir_json, bytes) else bir_json.encode()
        key = hashlib.sha256(bb).hexdigest()
        hit = cache.get(key)
        if hit is None:
            # persistent dir: the neff file is re-read on later cache hits
            hit = _orig(bir_json, tempfile.mkdtemp(), neff_name=neff_name)
            cache[key] = hit
        return hit

    b2j.compile_bir_kernel = _cached
    _cached(nc.to_json_bytes(), None)


def _build_warmup():
    """Tiny kernel exercising the collective path: absorbs one-time axon
    terminal init (device bring-up, global comm build) into untimed prep."""
    import concourse.mybir as mybir
    from concourse import bacc
    import concourse.tile as tile

    F32 = mybir.dt.float32
    nc = bacc.Bacc(None, target_bir_lowering=False)
    in_d = nc.dram_tensor("win", [128, 8], F32, kind="ExternalInput")
    out_d = nc.dram_tensor("wout", [128, 8], F32, kind="ExternalOutput")
    with tile.TileContext(nc) as tc:
        with tc.tile_pool(name="dram", bufs=1, space="DRAM") as dram:
            bin_ = dram.tile([128, 8], F32)
            agg = dram.tile([N_CORES, 128, 8], F32)
            rs = dram.tile([128, 8], F32)
            nc.gpsimd.dma_start(bin_[:], in_d[:])
            nc.gpsimd.collective_compute(
                "AllGather", mybir.AluOpType.bypass,
                replica_groups=[list(range(N_CORES))],
                ins=[bin_.opt()], outs=[agg.opt()])
            nc.gpsimd.collective_compute(
                "ReduceScatter", mybir.AluOpType.add,
                replica_groups=[list(range(N_CORES))],
                ins=[agg.opt()], outs=[rs.opt()])
            nc.gpsimd.dma_start(out_d[:], rs[:])
    nc.compile()
    return nc


def _pack_alibi(A_h):
    """[q, k] f32 head slice -> [128, AL_COLS] int8 causal-packed."""
    q8 = np.clip(np.rint(A_h.T * (1.0 / S_ALIBI)), -127, 127).astype(np.int8)
    T3 = np.ascontiguousarray(q8).reshape(EC, 128, SEQ)   # [ki, p, q]
    segs = []
    for qj in range(4):
        qs = slice(qj * SC, (qj + 1) * SC)
        if qj:
            segs.append(T3[:4 * qj, :, qs].transpose(1, 0, 2).reshape(128, -1))
        for t in range(4):
            segs.append(T3[4 * qj + t, :, qj * SC + t * 128:(qj + 1) * SC])
    return np.concatenate(segs, axis=1)


def _pack12(a16, H):
    """fp16 array -> (hi-byte plane, packed-nibble plane): 12-bit floats.

    Rounds to 12-bit mantissa, then pairs element j with j+H within each
    2H-block of the last dim (matching the device unpack's block slicing).
    """
    u = a16.view(np.uint16).astype(np.uint32)
    u12 = ((u + 8) & 0xFFF0).astype(np.uint16)
    hi = (u12 >> 8).astype(np.uint8)
    mid = ((u12 >> 4) & 0xF).astype(np.uint8)
    s = mid.shape
    m = mid.reshape(*s[:-1], s[-1] // (2 * H), 2, H)
    lo = ((m[..., 0, :] << 4) | m[..., 1, :]).reshape(*s[:-1], s[-1] // 2)
    return hi, lo


def _prep_inputs(x, alibi_bias, W_q, W_k, W_v, W_o):
    f16 = np.float16
    x = np.asarray(x, np.float32).reshape(BS, D_MODEL)
    # xT[e, s] -> [ec, p, s] fp16 -> 12-bit planes; core c ships ec [2c, 2c+2)
    xT = x.T.astype(f16).reshape(EC, 128, BS)
    xhi, xlo = _pack12(xT, SC // 2)
    xs_all = np.concatenate([xhi, xlo], axis=2)      # [EC, 128, 6144]

    scale = 1.0 / np.sqrt(np.float32(HEAD_DIM))

    in_maps = []
    for c in range(N_CORES):
        rows = slice(c * 256, (c + 1) * 256)

        def wt(W, s=1.0):
            # [e=2048, d_loc=256] -> [p, e_chunk, d] -> 12-bit hi||lo
            wT = (np.asarray(W, np.float32)[rows] * s).T
            w16 = np.ascontiguousarray(
                wT.reshape(EC, 128, 256).transpose(1, 0, 2).astype(f16))
            hi, lo = _pack12(w16, 128)
            return np.concatenate([hi, lo], axis=2)

        woT = np.asarray(W_o, np.float32)[:, rows].T      # [256, 2048]
        wo16 = np.ascontiguousarray(
            woT.reshape(H_LOC, 128, D_MODEL).transpose(1, 0, 2).astype(f16))
        # 8 segments of 512 (h-major), nibble pairs (j, j+256) within each
        whi, wlo = _pack12(wo16.reshape(128, 8, SC), SC // 2)

        alibi8 = np.stack([
            _pack_alibi(np.asarray(alibi_bias[2 * c + hl], np.float32))
            for hl in range(H_LOC)])

        in_maps.append({
            "xs": np.ascontiguousarray(xs_all[2 * c:2 * c + 2]),
            "wqT": wt(W_q, scale),
            "wkT": wt(W_k),
            "wvT": wt(W_v),
            "woT": np.concatenate([whi, wlo], axis=2),
            "alibi8": alibi8,
        })
    return in_maps


def kernel(x, alibi_bias, W_q, W_k, W_v, W_o, _trace=False):
    import time as _time
    from concourse.bass_utils import run_bass_kernel_spmd

    if "nc" not in _cache:
        _cache["nc"] = _build()
        _install_compile_cache(_cache["nc"])
    nc = _cache["nc"]

    t0 = _time.time()
    if not _cache.get("warmed"):
        wnc = _build_warmup()
        wmaps = [{"win": np.zeros((128, 8), np.float32)} for _ in range(N_CORES)]
        run_bass_kernel_spmd(wnc, wmaps, core_ids=list(range(N_CORES)))
        _cache["warmed"] = True
    in_maps = _prep_inputs(x, alibi_bias, W_q, W_k, W_v, W_o)
    _cache["prep_s"] = _time.time() - t0
    t0 = _time.time()
    res = run_bass_kernel_spmd(nc, in_maps, core_ids=list(range(N_CORES)),
                               trace=_trace)
    _cache["run_s"] = _time.time() - t0
    _cache["last_result"] = res
    y16 = np.concatenate(
        [np.asarray(om["y"], np.float16).reshape(SEQ // 4, D_MODEL)
         for om in res.results], axis=0)
    return y16.astype(np.float32).reshape(BATCH, SEQ, D_MODEL)
